# revision 3
# baseline (speedup 1.0000x reference)
# DeepGEMM-style fp8 block-quantized linear for Trainium2, 8-core SPMD.
#
# reference semantics:
#   x_dq = dequant(quant_e4m3fn(x, per-token per-128-group amax/448 scales))
#   w_dq = w_fp8 * w_scale (per 128x128 block)
#   out  = (x_dq @ w_dq.T).astype(bf16)          # fp32 accumulation
#
# Strategy (per core; 2x4 [M x N] grid => M2=2048, N2=1024 per core):
#   - TRN fp8_e4m3 tops out at 240 (vs OCP e4m3fn's 448), so quantize
#     x * (112/amax) on TRN's grid: identical rounding for normals (pure
#     exponent shift); dequantize with s4 = amax/112.
#   - scales folded into fp16 matmul operands (fp16 keeps the e4m3fn
#     values and 448-max weight values exact, and halves bf16's rounding
#     noise); on-chip xbar DMA transposes into [K, *] layouts.
#   - startup: the 29MB fp32 W stream takes ~82us at HBM bw, while the PE
#     eats a full m-tile's worth of W in ~24us.  So the first 4 m-tiles
#     are processed n-quarter-major (sweep), consuming each 256-wide W
#     quarter right as it lands; W is streamed in 32 small chunks with
#     the per-block scale applied pre-transpose, alternating V/G engines.
#   - x pipeline: DMA -> amax reduce (V) -> scale (G) -> recip (V) ->
#     fp8 quant (G) -> fp16 dequant (V, written in-place over the raw x
#     tile to save SBUF) -> xbar transpose to [K, m] tiles; 16-tile xt
#     pool holds 4 m-tiles so the sweep never starves.
#   - steady state (m-tiles 4..15): 512-wide psum tiles (full bank), two
#     per m-tile, x quantized 2 m-tiles ahead.

import numpy as np
import ml_dtypes
from contextlib import ExitStack

import concourse.bass as bass
import concourse.mybir as mybir
import concourse.tile as tile
from concourse import bacc
from concourse.bass_utils import run_bass_kernel_spmd

dt = mybir.dt

M, N, K = 4096, 4096, 7168
MSH, NSH = 2, 4                     # core grid: 2 along M, 4 along N
NCORES = MSH * NSH
BLK = 128


def bcast_inner(ap, n):
    """Append a step-0 inner dim of size n (free-dim broadcast read)."""
    return bass.AP(tensor=ap.tensor, offset=ap.offset, ap=[*ap.ap, [0, n]])


def emit_kernel(ctx, tc, o_d, x_d, w_d, ws_d):
    nc = tc.nc
    f32, f16, f8 = dt.float32, dt.float16, dt.float8e4
    bf16 = dt.bfloat16
    M2, Kd = x_d.shape
    N2, _ = w_d.shape
    KB = Kd // BLK              # 56 k-blocks
    NB = N2 // BLK              # 8 n-blocks
    MT = M2 // BLK              # 16 m-tiles
    KQ = 4                      # x pipeline chunks per m-tile
    KBQ = KB // KQ              # 14 k-blocks per x chunk
    KL = Kd // KQ               # 1792 cols per x chunk
    KHW = 4                     # w chunks per n-block
    KHL = Kd // KHW             # 1792 cols per w chunk
    KBH = KB // KHW             # 14 k-blocks per w chunk
    NWC = NB * KHW              # 32 w chunks total
    SW_MT = 4                   # m-tiles covered by the startup sweep
    NQ4 = N2 // 256             # 4 sweep quarters
    NH = N2 // 512              # 2 steady halves

    wtp = ctx.enter_context(tc.tile_pool(name="wt", bufs=1))
    constp = ctx.enter_context(tc.tile_pool(name="consts", bufs=1))
    wqp = ctx.enter_context(tc.tile_pool(name="wq", bufs=2))
    xnp = ctx.enter_context(tc.tile_pool(name="xn", bufs=2))
    scp = ctx.enter_context(tc.tile_pool(name="sc", bufs=4))
    xqp = ctx.enter_context(tc.tile_pool(name="xq", bufs=2))
    xtp = ctx.enter_context(tc.tile_pool(name="xt", bufs=4 * SW_MT))
    obp = ctx.enter_context(tc.tile_pool(name="ob", bufs=4))
    psap = ctx.enter_context(tc.tile_pool(name="psa", bufs=4, space="PSUM"))
    psbp = ctx.enter_context(tc.tile_pool(name="psb", bufs=3, space="PSUM"))

    # w_scale broadcast across partitions via step-0 partition DMA read
    wsb = constp.tile([128, NB * KB], f32)
    ws_flat = ws_d.rearrange("a b -> (a b)")
    ws_b = bass.AP(tensor=ws_flat.tensor, offset=ws_flat.offset,
                   ap=[[0, 128], *ws_flat.ap])
    nc.gpsimd.dma_start(wsb[:], ws_b)

    # persistent dequantized transposed weight: wt_t[p, kb, n]
    wt_t = wtp.tile([128, KB, N2], f16)

    def emit_w(c):
        """W chunk c (nb = c//KHW, kh = c%KHW): DMA f32->f16 cast, scale
        by w_scale pre-transpose (V/G alternating), xbar-transpose."""
        nb, kh = divmod(c, KHW)
        wq = wqp.tile([128, KHL], f16, tag="wq")
        nc.gpsimd.dma_start(
            wq[:], w_d[nb * BLK:(nb + 1) * BLK, kh * KHL:(kh + 1) * KHL])
        eng = nc.vector if c % 2 == 0 else nc.gpsimd
        wqg = wq[:].rearrange("p (kb c) -> p kb c", c=BLK)
        eng.tensor_tensor(
            out=wqg, in0=wqg,
            in1=bcast_inner(
                wsb[:, nb * KB + kh * KBH: nb * KB + (kh + 1) * KBH], BLK),
            op=mybir.AluOpType.mult)
        nc.sync.dma_start(
            wt_t[:, kh * KBH:(kh + 1) * KBH, nb * BLK:(nb + 1) * BLK],
            wq[:], transpose=True)

    xt_all = {}                 # (mt, q) -> transposed dequantized x tile

    def emit_x(mt, q):
        """x chunk (mt, q): DMA, amax, scales, fp8 quant, in-place fp16
        dequant, transpose to [k, kb, m]."""
        xn = xnp.tile([128, KL], f16, tag="xn")
        xn_b = xn[:].bitcast(bf16)
        nc.sync.dma_start(
            xn_b, x_d[mt * BLK:(mt + 1) * BLK, q * KL:(q + 1) * KL])
        xng = xn_b.rearrange("p (kb c) -> p kb c", c=BLK)

        amax = scp.tile([128, KBQ], f32, tag="amax")
        nc.vector.reduce_max(
            amax[:], xng, axis=mybir.AxisListType.X, apply_absolute_value=True)
        # s4 ~= max(amax, 1e-12)/112 (== 4x reference scale up to 1 ulp)
        s4 = scp.tile([128, KBQ], f32, tag="s4")
        nc.gpsimd.tensor_scalar(
            out=s4[:], in0=amax[:],
            scalar1=1e-12, scalar2=float(np.float32(1.0 / 112.0)),
            op0=mybir.AluOpType.max, op1=mybir.AluOpType.mult)
        inv4 = scp.tile([128, KBQ], f32, tag="inv4")
        nc.vector.reciprocal(inv4[:], s4[:])

        xq = xqp.tile([128, KL], f8, tag="xq")
        xqg = xq[:].rearrange("p (kb c) -> p kb c", c=BLK)
        nc.gpsimd.tensor_tensor(
            out=xqg, in0=xng, in1=bcast_inner(inv4[:], BLK),
            op=mybir.AluOpType.mult)
        # dequant back over the raw-x tile (f16 view of the same bytes)
        xdqg = xn[:].rearrange("p (kb c) -> p kb c", c=BLK)
        nc.vector.tensor_tensor(
            out=xdqg, in0=xqg, in1=bcast_inner(s4[:], BLK),
            op=mybir.AluOpType.mult)

        xt_t = xtp.tile([128, KBQ, 128], f16, tag="xt")
        nc.sync.dma_start(xt_t[:], xn[:], transpose=True)
        xt_all[(mt, q)] = xt_t

    def mm_group(mt, ps, n0, nw):
        for kb in range(KB):
            nc.tensor.matmul(
                ps[:],
                xt_all[(mt, kb // KBQ)][:, kb % KBQ, :],
                wt_t[:, kb, n0:n0 + nw],
                start=(kb == 0), stop=(kb == KB - 1))

    # ---- emission: W stream + first-4-m-tile x pipelines, interleaved so
    # each engine's FIFO roughly tracks HBM arrival order.
    for c in range(4):
        emit_w(c)
    wc = 4
    for i in range(SW_MT * KQ):
        emit_x(i // KQ, i % KQ)
        for _ in range(2):
            if wc < NWC:
                emit_w(wc)
                wc += 1
    # prefetch the first two steady-state m-tiles behind the sweep
    for mt in (4, 5):
        for q in range(KQ):
            emit_x(mt, q)

    # ---- startup sweep: (mt, nq) pairs in W/x-arrival-feasible order
    SWEEP = [(0, 0), (1, 0), (0, 1), (1, 1), (2, 0), (2, 1), (0, 2), (1, 2),
             (2, 2), (3, 0), (3, 1), (0, 3), (1, 3), (2, 3), (3, 2), (3, 3)]
    obs = {mt: obp.tile([128, N2], bf16, tag="ob", name=f"ob{mt}")
           for mt in range(SW_MT)}
    done = {mt: 0 for mt in range(SW_MT)}
    for mt, nq in SWEEP:
        ps = psap.tile([128, 256], f32, tag="psa")
        mm_group(mt, ps, nq * 256, 256)
        nc.scalar.copy(obs[mt][:, nq * 256:(nq + 1) * 256], ps[:])
        done[mt] += 1
        if done[mt] == NQ4:
            nc.sync.dma_start(o_d[mt * BLK:(mt + 1) * BLK, :], obs[mt][:])

    # ---- steady state: full-bank 512-wide psum tiles, 2-m-tile lookahead
    for mt in range(SW_MT, MT):
        la = mt + 2
        if la < MT:
            for q in range(KQ):
                emit_x(la, q)
        ob = obp.tile([128, N2], bf16, tag="ob")
        for h in range(NH):
            ps = psbp.tile([128, 512], f32, tag="psb")
            mm_group(mt, ps, h * 512, 512)
            nc.scalar.copy(ob[:, h * 512:(h + 1) * 512], ps[:])
        nc.sync.dma_start(o_d[mt * BLK:(mt + 1) * BLK, :], ob[:])


def build_nc(m2, n2, k, **kw):
    nc = bacc.Bacc("TRN2", target_bir_lowering=False, debug=False, num_devices=NCORES)
    x_d = nc.dram_tensor("x", [m2, k], dt.bfloat16, kind="ExternalInput").ap()
    w_d = nc.dram_tensor("w", [n2, k], dt.float32, kind="ExternalInput").ap()
    ws_d = nc.dram_tensor("ws", [n2 // BLK, k // BLK], dt.float32, kind="ExternalInput").ap()
    o_d = nc.dram_tensor("o", [m2, n2], dt.bfloat16, kind="ExternalOutput").ap()
    with tile.TileContext(nc) as tc, ExitStack() as ctx:
        emit_kernel(ctx, tc, o_d, x_d, w_d, ws_d, **kw)
    nc.compile()
    return nc


_cache = {}


def _get_nc():
    if "nc" not in _cache:
        _cache["nc"] = build_nc(M // MSH, N // NSH, K)
    return _cache["nc"]


def kernel(input, weight_fp8, weight_scale, _trace=False, _trace_kwargs=None):
    input = np.asarray(input)
    if input.dtype != ml_dtypes.bfloat16:
        input = input.astype(ml_dtypes.bfloat16)
    weight_fp8 = np.asarray(weight_fp8, dtype=np.float32)
    weight_scale = np.asarray(weight_scale, dtype=np.float32)
    M2, N2 = M // MSH, N // NSH
    NSB = N2 // BLK

    in_maps = []
    for c in range(NCORES):
        mi, ni = divmod(c, NSH)
        in_maps.append({
            "x": np.ascontiguousarray(input[mi * M2:(mi + 1) * M2]),
            "w": np.ascontiguousarray(weight_fp8[ni * N2:(ni + 1) * N2]),
            "ws": np.ascontiguousarray(weight_scale[ni * NSB:(ni + 1) * NSB]),
        })

    nc = _get_nc()
    kw = {}
    if _trace:
        kw = dict(trace=True, **(_trace_kwargs or {}))
    res = run_bass_kernel_spmd(nc, in_maps, core_ids=list(range(NCORES)), **kw)

    out = np.empty((M, N), dtype=ml_dtypes.bfloat16)
    for c in range(NCORES):
        mi, ni = divmod(c, NSH)
        out[mi * M2:(mi + 1) * M2, ni * N2:(ni + 1) * N2] = res.results[c]["o"]
    if _trace:
        return out, res
    return out


# revision 8
# speedup vs baseline: 1.0643x; 1.0643x over previous
# DeepGEMM-style fp8 block-quantized linear for Trainium2, 8-core SPMD.
#
# reference semantics:
#   x_dq = dequant(quant_e4m3fn(x, per-token per-128-group amax/448 scales))
#   w_dq = w_fp8 * w_scale (per 128x128 block)
#   out  = (x_dq @ w_dq.T).astype(bf16)          # fp32 accumulation
#
# Strategy (per core; 2x4 [M x N] grid => M2=2048, N2=1024 per core):
#   - TRN fp8_e4m3 tops out at 240 (vs OCP e4m3fn's 448), so quantize
#     x * (112/amax) on TRN's grid: identical rounding for normals (pure
#     exponent shift); dequantize with s4 = amax/112.
#   - scales folded into fp16 matmul operands (fp16 keeps the e4m3fn
#     values and 448-max weight values exact, and halves bf16's rounding
#     noise); on-chip xbar DMA transposes into [K, *] layouts.
#   - startup: the 29MB fp32 W stream takes ~82us at HBM bw, while the PE
#     eats a full m-tile's worth of W in ~24us.  So the first 4 m-tiles
#     are processed n-quarter-major (sweep), consuming each 256-wide W
#     quarter right as it lands; W is streamed in 32 small chunks with
#     the per-block scale applied pre-transpose, alternating V/G engines.
#   - x pipeline: DMA -> amax reduce (V) -> scale (G) -> recip (V) ->
#     fp8 quant (G) -> fp16 dequant (V, written in-place over the raw x
#     tile to save SBUF) -> xbar transpose to [K, m] tiles; 16-tile xt
#     pool holds 4 m-tiles so the sweep never starves.
#   - steady state (m-tiles 4..15): 512-wide psum tiles (full bank), two
#     per m-tile, x quantized 2 m-tiles ahead.

import numpy as np
import ml_dtypes
from contextlib import ExitStack

import concourse.bass as bass
import concourse.mybir as mybir
import concourse.tile as tile
from concourse import bacc
from concourse.bass_utils import run_bass_kernel_spmd

dt = mybir.dt

M, N, K = 4096, 4096, 7168
MSH, NSH = 2, 4                     # core grid: 2 along M, 4 along N
NCORES = MSH * NSH
BLK = 128


def bcast_inner(ap, n):
    """Append a step-0 inner dim of size n (free-dim broadcast read)."""
    return bass.AP(tensor=ap.tensor, offset=ap.offset, ap=[*ap.ap, [0, n]])


def emit_kernel(ctx, tc, o_d, x_d, w_d, ws_d):
    nc = tc.nc
    f32, f16, f8 = dt.float32, dt.float16, dt.float8e4
    bf16 = dt.bfloat16
    M2, Kd = x_d.shape
    N2, _ = w_d.shape
    KB = Kd // BLK              # 56 k-blocks
    NB = N2 // BLK              # 8 n-blocks
    MT = M2 // BLK              # 16 m-tiles
    KQ = 4                      # x pipeline chunks per m-tile
    KBQ = KB // KQ              # 14 k-blocks per x chunk
    KL = Kd // KQ               # 1792 cols per x chunk
    KHW = 4                     # w chunks per n-block
    KHL = Kd // KHW             # 1792 cols per w chunk
    KBH = KB // KHW             # 14 k-blocks per w chunk
    NWC = NB * KHW              # 32 w chunks total
    SW_MT = 4                   # m-tiles covered by the startup sweep
    NQ4 = N2 // 256             # 4 sweep quarters
    NH = N2 // 512              # 2 steady halves

    wtp = ctx.enter_context(tc.tile_pool(name="wt", bufs=1))
    constp = ctx.enter_context(tc.tile_pool(name="consts", bufs=1))
    wqp = ctx.enter_context(tc.tile_pool(name="wq", bufs=2))
    xnp = ctx.enter_context(tc.tile_pool(name="xn", bufs=3))
    scp = ctx.enter_context(tc.tile_pool(name="sc", bufs=3))
    xqp = ctx.enter_context(tc.tile_pool(name="xq", bufs=2))
    xdqp = ctx.enter_context(tc.tile_pool(name="xdq", bufs=2))
    xtp = ctx.enter_context(tc.tile_pool(name="xt", bufs=4 * SW_MT))
    obp = ctx.enter_context(tc.tile_pool(name="ob", bufs=4))
    psap = ctx.enter_context(tc.tile_pool(name="psa", bufs=4, space="PSUM"))
    psbp = ctx.enter_context(tc.tile_pool(name="psb", bufs=3, space="PSUM"))

    # w_scale broadcast across partitions via step-0 partition DMA read
    wsb = constp.tile([128, NB * KB], f32)
    ws_flat = ws_d.rearrange("a b -> (a b)")
    ws_b = bass.AP(tensor=ws_flat.tensor, offset=ws_flat.offset,
                   ap=[[0, 128], *ws_flat.ap])
    nc.gpsimd.dma_start(wsb[:], ws_b)

    # persistent dequantized transposed weight: wt_t[p, kb, n]
    wt_t = wtp.tile([128, KB, N2], f16)

    def emit_w(c):
        """W chunk c (nb = c//KHW, kh = c%KHW): DMA f32->f16 cast,
        xbar-transpose (frees wq so the W stream runs at HBM rate),
        then scale by w_scale in wt_t (V/G alternating)."""
        nb, kh = divmod(c, KHW)
        wq = wqp.tile([128, KHL], f16, tag="wq")
        nc.gpsimd.dma_start(
            wq[:], w_d[nb * BLK:(nb + 1) * BLK, kh * KHL:(kh + 1) * KHL])
        sl = wt_t[:, kh * KBH:(kh + 1) * KBH, nb * BLK:(nb + 1) * BLK]
        nc.sync.dma_start(sl, wq[:], transpose=True)
        eng = nc.vector if c % 3 < 2 else nc.gpsimd
        eng.tensor_tensor(
            out=sl, in0=sl,
            in1=bcast_inner(
                wsb[:, nb * KB + kh * KBH: nb * KB + (kh + 1) * KBH], BLK),
            op=mybir.AluOpType.mult)

    xt_all = {}                 # (mt, q) -> transposed dequantized x tile

    def emit_x(mt, q):
        """x chunk (mt, q): DMA, amax, scales, fp8 quant, fp16 dequant,
        transpose to [k, kb, m]."""
        xn = xnp.tile([128, KL], bf16, tag="xn")
        nc.sync.dma_start(
            xn[:], x_d[mt * BLK:(mt + 1) * BLK, q * KL:(q + 1) * KL])
        xng = xn[:].rearrange("p (kb c) -> p kb c", c=BLK)

        amax = scp.tile([128, KBQ], f32, tag="amax")
        nc.vector.reduce_max(
            amax[:], xng, axis=mybir.AxisListType.X, apply_absolute_value=True)
        # s4 ~= max(amax, 1e-12)/112 (== 4x reference scale up to 1 ulp)
        s4 = scp.tile([128, KBQ], f32, tag="s4")
        nc.vector.tensor_scalar(
            out=s4[:], in0=amax[:],
            scalar1=1e-12, scalar2=float(np.float32(1.0 / 112.0)),
            op0=mybir.AluOpType.max, op1=mybir.AluOpType.mult)
        inv4 = scp.tile([128, KBQ], f32, tag="inv4")
        nc.vector.reciprocal(inv4[:], s4[:])

        xq = xqp.tile([128, KL], f8, tag="xq")
        xqg = xq[:].rearrange("p (kb c) -> p kb c", c=BLK)
        nc.gpsimd.tensor_tensor(
            out=xqg, in0=xng, in1=bcast_inner(inv4[:], BLK),
            op=mybir.AluOpType.mult)
        xdq = xdqp.tile([128, KL], f16, tag="xdq")
        xdqg = xdq[:].rearrange("p (kb c) -> p kb c", c=BLK)
        nc.vector.tensor_tensor(
            out=xdqg, in0=xqg, in1=bcast_inner(s4[:], BLK),
            op=mybir.AluOpType.mult)

        xt_t = xtp.tile([128, KBQ, 128], f16, tag="xt")
        nc.sync.dma_start(xt_t[:], xdq[:], transpose=True)
        xt_all[(mt, q)] = xt_t

    def mm_group(mt, ps, n0, nw):
        for kb in range(KB):
            nc.tensor.matmul(
                ps[:],
                xt_all[(mt, kb // KBQ)][:, kb % KBQ, :],
                wt_t[:, kb, n0:n0 + nw],
                start=(kb == 0), stop=(kb == KB - 1))

    # ---- emission: W stream + first-4-m-tile x pipelines, interleaved so
    # each engine's FIFO roughly tracks HBM arrival order.
    for c in range(4):
        emit_w(c)
    wc = 4
    for i in range(SW_MT * KQ):
        emit_x(i // KQ, i % KQ)
        for _ in range(2):
            if wc < NWC:
                emit_w(wc)
                wc += 1
    # prefetch the first two steady-state m-tiles behind the sweep
    for mt in (4, 5):
        for q in range(KQ):
            emit_x(mt, q)

    # ---- startup sweep: (mt, nq) pairs in W/x-arrival-feasible order
    SWEEP = [(0, 0), (1, 0), (0, 1), (1, 1), (2, 0), (2, 1), (0, 2), (1, 2),
             (2, 2), (3, 0), (3, 1), (0, 3), (1, 3), (2, 3), (3, 2), (3, 3)]
    obs = {mt: obp.tile([128, N2], bf16, tag="ob", name=f"ob{mt}")
           for mt in range(SW_MT)}
    done = {mt: 0 for mt in range(SW_MT)}
    for mt, nq in SWEEP:
        ps = psap.tile([128, 256], f32, tag="psa")
        mm_group(mt, ps, nq * 256, 256)
        nc.scalar.copy(obs[mt][:, nq * 256:(nq + 1) * 256], ps[:])
        done[mt] += 1
        if done[mt] == NQ4:
            nc.sync.dma_start(o_d[mt * BLK:(mt + 1) * BLK, :], obs[mt][:])

    # ---- steady state: full-bank 512-wide psum tiles, 2-m-tile lookahead
    for mt in range(SW_MT, MT):
        la = mt + 2
        if la < MT:
            for q in range(KQ):
                emit_x(la, q)
        ob = obp.tile([128, N2], bf16, tag="ob")
        for h in range(NH):
            ps = psbp.tile([128, 512], f32, tag="psb")
            mm_group(mt, ps, h * 512, 512)
            nc.scalar.copy(ob[:, h * 512:(h + 1) * 512], ps[:])
        nc.sync.dma_start(o_d[mt * BLK:(mt + 1) * BLK, :], ob[:])


def build_nc(m2, n2, k, **kw):
    nc = bacc.Bacc("TRN2", target_bir_lowering=False, debug=False, num_devices=NCORES)
    x_d = nc.dram_tensor("x", [m2, k], dt.bfloat16, kind="ExternalInput").ap()
    w_d = nc.dram_tensor("w", [n2, k], dt.float32, kind="ExternalInput").ap()
    ws_d = nc.dram_tensor("ws", [n2 // BLK, k // BLK], dt.float32, kind="ExternalInput").ap()
    o_d = nc.dram_tensor("o", [m2, n2], dt.bfloat16, kind="ExternalOutput").ap()
    with tile.TileContext(nc) as tc, ExitStack() as ctx:
        emit_kernel(ctx, tc, o_d, x_d, w_d, ws_d, **kw)
    nc.compile()
    return nc


_cache = {}


def _get_nc():
    if "nc" not in _cache:
        _cache["nc"] = build_nc(M // MSH, N // NSH, K)
    return _cache["nc"]


def kernel(input, weight_fp8, weight_scale, _trace=False, _trace_kwargs=None):
    input = np.asarray(input)
    if input.dtype != ml_dtypes.bfloat16:
        input = input.astype(ml_dtypes.bfloat16)
    weight_fp8 = np.asarray(weight_fp8, dtype=np.float32)
    weight_scale = np.asarray(weight_scale, dtype=np.float32)
    M2, N2 = M // MSH, N // NSH
    NSB = N2 // BLK

    in_maps = []
    for c in range(NCORES):
        mi, ni = divmod(c, NSH)
        in_maps.append({
            "x": np.ascontiguousarray(input[mi * M2:(mi + 1) * M2]),
            "w": np.ascontiguousarray(weight_fp8[ni * N2:(ni + 1) * N2]),
            "ws": np.ascontiguousarray(weight_scale[ni * NSB:(ni + 1) * NSB]),
        })

    nc = _get_nc()
    kw = {}
    if _trace:
        kw = dict(trace=True, **(_trace_kwargs or {}))
    res = run_bass_kernel_spmd(nc, in_maps, core_ids=list(range(NCORES)), **kw)

    out = np.empty((M, N), dtype=ml_dtypes.bfloat16)
    for c in range(NCORES):
        mi, ni = divmod(c, NSH)
        out[mi * M2:(mi + 1) * M2, ni * N2:(ni + 1) * N2] = res.results[c]["o"]
    if _trace:
        return out, res
    return out
